# revision 34
# baseline (speedup 1.0000x reference)
"""Trainium2 Bass kernel for IntegralTransform GNN message passing.

Strategy (dst-sharded, 8 cores, V5):
  - Node space padded to 50176 = 8 * 49 * 128. Core c owns nodes
    [c*6272, (c+1)*6272) = 49 buckets of 128 nodes. Host bins edges by dst
    bucket (stable sort) and sorts each core's buckets by edge count
    (descending) into 49 SLOTS; slot b's chunk count k_b is the max across
    the 8 cores, so one SPMD program fits all cores with minimal padding
    (~814 chunks/core vs 882 fixed). Host gathers pos[src]|pos[dst] and
    x[src] per slot, precomputes the per-chunk one-hot scatter matrices
    (bf16), and adds the Wo-bias term (xs_agg @ bo) after the device
    returns the pure message aggregation.
  - Edge stream is laid out two-chunks-deep: even chunks live on SBUF
    partitions 0-63, odd chunks on 64-127. L1/L2 run as two concurrent
    PE sub-tiles (col/row groups), so gelu processes 128 partitions per
    op. L3 runs one matmul per chunk PAIR: the two-deep h2 column block
    [128,128] is exactly the stacked operand for a block-diagonal
    rhs [[Wo,0],[0,Wo]] [128,512], yielding hp for both chunks in one
    [128,512] PSUM bank (one LDWEIGHTS per two chunks). Odd slot sizes
    get a single-chunk tail matmul.
  - DVE multiplies each hp pair by xs (broadcast along o) in ONE op
    straight out of PSUM -> prod bf16 [128, 2, 16, 16]. Scatter is
    transposed: acc2T[n, (o,i)] += oh_c^T @ prod_c -- one N=256 matmul
    and one LDWEIGHTS (the one-hot) per chunk, a single per-slot
    accumulation group in one PSUM bank. A per-slot DVE tensor_reduce
    over i produces out[n, o] directly; one [128, 49*16] DMA returns it.
    No collectives; host un-permutes slots, concatenates, adds the bias.
"""

import numpy as np
import ml_dtypes

N_POINTS = 50000
N_PAD = 50176          # 8 * 49 * 128
N_CORES = 8
BUCKET = 128           # nodes per bucket
B_PER_CORE = 49
N_BUCKETS = N_PAD // BUCKET   # 392
CORE_NODES = B_PER_CORE * BUCKET  # 6272
IN_CH = 16
OUT_CH = 16
HID = 64
POS = 3

BF16 = ml_dtypes.bfloat16

_PROGRAM_CACHE = {}


def _build_program(ks):
    """Build + compile the per-core Bass program.

    ks = tuple of 49 per-slot chunk counts (shared across cores)."""
    import concourse.bacc as bacc
    import concourse.tile as tile
    import concourse.mybir as mybir

    f32 = mybir.dt.float32
    bf16 = mybir.dt.bfloat16

    nblks = [(k + 1) // 2 for k in ks]       # 128-col blocks in two-deep layout
    hoffs = np.concatenate([[0], np.cumsum([n * 128 for n in nblks])])
    coffs = np.concatenate([[0], np.cumsum(ks)])
    total_half = int(hoffs[-1])
    total_chunks = int(coffs[-1])
    Kmax = max(ks)
    Hmax = max(nblks) * 128

    nc = bacc.Bacc("TRN2", target_bir_lowering=False, debug=False)

    PT = nc.dram_tensor("PT", [12, total_half], bf16, kind="ExternalInput")
    XSB = nc.dram_tensor("XSB", [128, total_chunks * 16], bf16,
                         kind="ExternalInput")
    OH = nc.dram_tensor("OH", [128, total_chunks * 128], bf16,
                        kind="ExternalInput")
    W12 = nc.dram_tensor("W12", [128, HID], bf16, kind="ExternalInput")
    B1 = nc.dram_tensor("B1", [128, 1], f32, kind="ExternalInput")
    WH2 = nc.dram_tensor("WH2", [128, HID], bf16, kind="ExternalInput")
    BH = nc.dram_tensor("BH", [128, 1], f32, kind="ExternalInput")
    WOD = nc.dram_tensor("WOD", [128, 512], bf16, kind="ExternalInput")
    OUT = nc.dram_tensor("OUT", [128, B_PER_CORE * 16], f32,
                         kind="ExternalOutput")

    Gelu = mybir.ActivationFunctionType.Gelu
    MUL = mybir.AluOpType.mult
    ADD = mybir.AluOpType.add
    X = mybir.AxisListType.X

    with tile.TileContext(nc) as tc:
        with tc.tile_pool(name="const", bufs=1) as cp, \
             tc.tile_pool(name="io", bufs=2) as io, \
             tc.tile_pool(name="hh", bufs=2) as hh, \
             tc.tile_pool(name="wk", bufs=10) as wk, \
             tc.tile_pool(name="psMLP", bufs=2, space="PSUM") as psMLP, \
             tc.tile_pool(name="psH", bufs=4, space="PSUM") as psH, \
             tc.tile_pool(name="psAcc", bufs=2, space="PSUM") as psAcc:

            # --- constants ---
            w12_t = cp.tile([128, HID], bf16)
            nc.sync.dma_start(out=w12_t[:], in_=W12[:])
            b1_t = cp.tile([128, 1], f32)
            nc.sync.dma_start(out=b1_t[:], in_=B1[:])
            wh2_t = cp.tile([128, HID], bf16)
            nc.sync.dma_start(out=wh2_t[:], in_=WH2[:])
            bh_t = cp.tile([128, 1], f32)
            nc.sync.dma_start(out=bh_t[:], in_=BH[:])
            wod_t = cp.tile([128, 512], bf16)
            nc.sync.dma_start(out=wod_t[:], in_=WOD[:])

            fin_t = cp.tile([128, B_PER_CORE * 16], f32)

            for b in range(B_PER_CORE):
                k = ks[b]
                npairs, tail = k // 2, k % 2
                nblk = npairs + tail
                Hb = nblk * 128
                ho = int(hoffs[b])
                co = int(coffs[b])

                pt_t = io.tile([128, Hmax], bf16, tag="pt")
                nc.sync.dma_start(out=pt_t[0:6, 0:Hb],
                                  in_=PT[0:6, ho:ho + Hb])
                nc.sync.dma_start(out=pt_t[64:70, 0:Hb],
                                  in_=PT[6:12, ho:ho + Hb])
                xsb_t = io.tile([128, Kmax, 16], bf16, tag="xsb")
                nc.sync.dma_start(out=xsb_t[:, 0:k, :],
                                  in_=XSB[:, co * 16:(co + k) * 16])
                oh_t = io.tile([128, Kmax * 128], bf16, tag="oh")
                nc.sync.dma_start(out=oh_t[:, 0:k * 128],
                                  in_=OH[:, co * 128:(co + k) * 128])

                # ---- L1: two concurrent col-group tiles, gelu on 128p ----
                h1_t = hh.tile([128, Hmax], bf16, tag="h1")
                h2_t = hh.tile([128, Hmax], bf16, tag="h2")
                for s0 in range(0, Hb, 512):
                    w = min(512, Hb - s0)
                    p1 = psMLP.tile([128, 512], f32, tag="ps_mlp")
                    nc.tensor.matmul(p1[0:64, 0:w], lhsT=w12_t[0:6, :],
                                     rhs=pt_t[0:6, s0:s0 + w],
                                     start=True, stop=True)
                    nc.tensor.matmul(p1[64:128, 0:w], lhsT=w12_t[64:70, :],
                                     rhs=pt_t[64:70, s0:s0 + w],
                                     start=True, stop=True)
                    nc.scalar.activation(h1_t[:, s0:s0 + w], p1[:, 0:w], Gelu,
                                         bias=b1_t[:], scale=1.0)
                # ---- L2 slices emitted lazily inside the pair loop ----
                def emit_l2(s0):
                    w = min(512, Hb - s0)
                    p2 = psMLP.tile([128, 512], f32, tag="ps_mlp")
                    nc.tensor.matmul(p2[0:64, 0:w], lhsT=wh2_t[0:64, :],
                                     rhs=h1_t[0:64, s0:s0 + w],
                                     start=True, stop=True)
                    nc.tensor.matmul(p2[64:128, 0:w], lhsT=wh2_t[64:128, :],
                                     rhs=h1_t[64:128, s0:s0 + w],
                                     start=True, stop=True)
                    nc.scalar.activation(h2_t[:, s0:s0 + w], p2[:, 0:w], Gelu,
                                         bias=bh_t[:], scale=1.0)
                l2_next = [0]

                # ---- per-pair hp (block-diag) + mult + skewed scatter ----
                acc2 = psAcc.tile([128, 256], f32, tag="acc2")
                pend = []

                def emit_scatter(chunks, prod2, first, last):
                    pr = prod2[:].rearrange("p c o i -> p (c o i)")
                    for idx, c in enumerate(chunks):
                        nc.tensor.matmul(
                            acc2[:], lhsT=oh_t[:, c * 128:(c + 1) * 128],
                            rhs=pr[:, idx * 256:(idx + 1) * 256],
                            start=(first and idx == 0),
                            stop=(last and idx == len(chunks) - 1))

                for m in range(nblk):
                    # h2 cols [m*128, (m+1)*128) need L2 slice m//4 (+1 ahead)
                    while l2_next[0] < Hb and l2_next[0] <= (m + 1) * 128:
                        emit_l2(l2_next[0])
                        l2_next[0] += 512
                    is_tail = (tail == 1 and m == nblk - 1)
                    hpP = psH.tile([128, 512], f32, tag="hpP")
                    prod2 = wk.tile([128, 2, 16, 16], bf16, tag="prod")
                    if not is_tail:
                        nc.tensor.matmul(hpP[:],
                                         lhsT=h2_t[:, m * 128:(m + 1) * 128],
                                         rhs=wod_t[:], start=True, stop=True)
                        xs_b = xsb_t[:, 2 * m:2 * m + 2, :].unsqueeze(
                            2).to_broadcast([128, 2, 16, 16])
                        nc.vector.tensor_tensor(
                            out=prod2[:],
                            in0=hpP[:].rearrange("p (c o i) -> p c o i",
                                                 c=2, o=16, i=16),
                            in1=xs_b, op=MUL)
                        chunks = [2 * m, 2 * m + 1]
                    else:
                        nc.tensor.matmul(hpP[:, 0:256],
                                         lhsT=h2_t[0:64, m * 128:(m + 1) * 128],
                                         rhs=wod_t[0:64, 0:256],
                                         start=True, stop=True)
                        xs_b = xsb_t[:, k - 1:k, :].unsqueeze(
                            2).to_broadcast([128, 1, 16, 16])
                        nc.vector.tensor_tensor(
                            out=prod2[:, 0:1],
                            in0=hpP[:, 0:256].rearrange(
                                "p (c o i) -> p c o i", c=1, o=16, i=16),
                            in1=xs_b, op=MUL)
                        chunks = [k - 1]
                    pend.append((chunks, prod2, m == 0))
                for j, e in enumerate(pend):
                    emit_scatter(e[0], e[1], e[2], j == len(pend) - 1)

                # out[n, o] = sum_i acc2T[n, (o,i)]
                nc.vector.tensor_reduce(
                    out=fin_t[:, b * 16:(b + 1) * 16],
                    in_=acc2[:].rearrange("p (o i) -> p o i", o=16, i=16),
                    axis=X, op=ADD)

            nc.sync.dma_start(out=OUT[:], in_=fin_t[:])

    nc.compile()
    return nc


def _host_prep(x, pos, edge_index, W1, b1, Wh, bh, Wo, bo):
    """Bin edges by dst bucket, sort buckets into slots, gather, pad."""
    x_flat = np.ascontiguousarray(x.reshape(-1, IN_CH).astype(np.float32))
    pos = np.ascontiguousarray(pos.astype(np.float32))
    src = np.asarray(edge_index[0], dtype=np.int64)
    dst = np.asarray(edge_index[1], dtype=np.int64)
    E = src.shape[0]

    bucket = (dst >> 7).astype(np.int64)          # 0..391
    order = np.argsort(bucket, kind="stable")     # edge ids sorted by bucket
    cnt = np.bincount(bucket, minlength=N_BUCKETS)
    starts = np.zeros(N_BUCKETS, dtype=np.int64)
    starts[1:] = np.cumsum(cnt)[:-1]

    cnt_pc = cnt.reshape(N_CORES, B_PER_CORE)
    perms = np.argsort(-cnt_pc, axis=1, kind="stable")   # slot -> local bucket
    sorted_cnt = np.take_along_axis(cnt_pc, perms, axis=1)
    slot_max = sorted_cnt.max(axis=0)
    ks = tuple(int(v) for v in np.maximum(1, np.ceil(slot_max / 128))
               .astype(np.int64))

    nblks = [(k + 1) // 2 for k in ks]
    hoffs = np.concatenate([[0], np.cumsum([n * 128 for n in nblks])])
    coffs = np.concatenate([[0], np.cumsum(ks)])
    total_half = int(hoffs[-1])
    total_chunks = int(coffs[-1])

    # host-side bias term
    e_src_all = src[order]
    e_dst_all = dst[order]
    xs_agg = np.zeros((N_PAD, IN_CH), dtype=np.float32)
    np.add.at(xs_agg, e_dst_all, x_flat[e_src_all])
    bo16 = np.asarray(bo, dtype=np.float32).reshape(IN_CH, OUT_CH)
    bias_full = xs_agg @ bo16                      # [N_PAD, 16]

    per_core = []
    for c in range(N_CORES):
        PT2 = np.zeros((12, total_half), dtype=np.float32)
        XS2 = np.zeros((128, total_chunks, 16), dtype=np.float32)
        OH2 = np.zeros((128, total_chunks, 128), dtype=BF16)
        for b in range(B_PER_CORE):
            k = ks[b]
            nblk = nblks[b]
            ho = int(hoffs[b])
            g = c * B_PER_CORE + int(perms[c][b])  # global bucket id
            n = int(cnt_pc[c][perms[c][b]])
            if n == 0:
                continue
            eids = order[starts[g]:starts[g] + n]
            es, ed = src[eids], dst[eids]
            pe6 = np.concatenate([pos[es], pos[ed]], axis=1)   # [n, 6]
            xse = x_flat[es]                                   # [n, 16]
            dl = (ed - (g << 7)).astype(np.int64)
            ch = np.arange(n) // 128                           # chunk in slot
            rw = np.arange(n) % 128                            # row (edge lane)
            # PT two-deep: chunk 2m -> rows 0-5 block m; 2m+1 -> rows 6-11
            colh = (ch // 2) * 128 + rw
            hi = (ch % 2) * 6
            PT2[hi, ho + colh] = pe6[:, 0]
            PT2[hi + 1, ho + colh] = pe6[:, 1]
            PT2[hi + 2, ho + colh] = pe6[:, 2]
            PT2[hi + 3, ho + colh] = pe6[:, 3]
            PT2[hi + 4, ho + colh] = pe6[:, 4]
            PT2[hi + 5, ho + colh] = pe6[:, 5]
            XS2[rw, int(coffs[b]) + ch] = xse
            OH2[rw, int(coffs[b]) + ch, dl] = 1
        per_core.append({
            "PT": PT2.astype(BF16),
            "XSB": np.ascontiguousarray(XS2.reshape(128, total_chunks * 16)
                                        ).astype(BF16),
            "OH": np.ascontiguousarray(OH2.reshape(128, total_chunks * 128)),
        })

    # weights (shared across cores)
    W1a = np.asarray(W1, dtype=BF16)                                # [6, 64]
    W12 = np.zeros((128, HID), dtype=BF16)
    W12[0:6] = W1a
    W12[64:70] = W1a
    b1a = np.tile(np.asarray(b1, dtype=np.float32).reshape(HID, 1), (2, 1))
    Wha = np.asarray(Wh, dtype=BF16)                                # [64, 64]
    Wh2 = np.vstack([Wha, Wha])                                     # [128, 64]
    bha = np.tile(np.asarray(bh, dtype=np.float32).reshape(HID, 1), (2, 1))
    WoP = np.asarray(Wo, dtype=np.float32).reshape(HID, IN_CH, OUT_CH)
    WoP = np.ascontiguousarray(WoP.transpose(0, 2, 1)).reshape(HID, 256)
    WoP = WoP.astype(BF16)                                          # [64,(o,i)]
    WoD = np.zeros((128, 512), dtype=BF16)
    WoD[0:64, 0:256] = WoP
    WoD[64:128, 256:512] = WoP
    shared = {"W12": W12, "B1": b1a, "WH2": Wh2, "BH": bha, "WOD": WoD}
    for m in per_core:
        m.update(shared)
    return ks, perms, per_core, bias_full


def kernel(**inputs):
    from concourse import bass_utils

    ks, perms, in_maps, bias_full = _host_prep(
        inputs["x"], inputs["pos"], inputs["edge_index"],
        inputs["W1"], inputs["b1"], inputs["Wh"], inputs["bh"],
        inputs["Wo"], inputs["bo"])

    if ks not in _PROGRAM_CACHE:
        _PROGRAM_CACHE[ks] = _build_program(ks)
    nc = _PROGRAM_CACHE[ks]

    res = bass_utils.run_bass_kernel_spmd(nc, in_maps,
                                          core_ids=list(range(N_CORES)))
    cores = []
    for c, r in enumerate(res.results):
        o = r["OUT"]                                   # [128, 49*16] slot-major
        o = o.reshape(128, B_PER_CORE, OUT_CH).transpose(1, 0, 2)
        core_out = np.empty((B_PER_CORE, 128, OUT_CH), dtype=np.float32)
        core_out[perms[c]] = o                         # un-permute slots
        cores.append(core_out.reshape(CORE_NODES, OUT_CH))
    out = np.concatenate(cores, axis=0)                # [50176, 16]
    out = out + bias_full
    return np.ascontiguousarray(
        out[:N_POINTS].reshape(1, N_POINTS, OUT_CH).astype(np.float32))


# revision 35
# speedup vs baseline: 1.0028x; 1.0028x over previous
"""Trainium2 Bass kernel for IntegralTransform GNN message passing.

Strategy (dst-sharded, 8 cores, V5):
  - Node space padded to 50176 = 8 * 49 * 128. Core c owns nodes
    [c*6272, (c+1)*6272) = 49 buckets of 128 nodes. Host bins edges by dst
    bucket (stable sort) and sorts each core's buckets by edge count
    (descending) into 49 SLOTS; slot b's chunk count k_b is the max across
    the 8 cores, so one SPMD program fits all cores with minimal padding
    (~814 chunks/core vs 882 fixed). Host gathers pos[src]|pos[dst] and
    x[src] per slot, precomputes the per-chunk one-hot scatter matrices
    (bf16), and adds the Wo-bias term (xs_agg @ bo) after the device
    returns the pure message aggregation.
  - Edge stream is laid out two-chunks-deep: even chunks live on SBUF
    partitions 0-63, odd chunks on 64-127. L1/L2 run as two concurrent
    PE sub-tiles (col/row groups), so gelu processes 128 partitions per
    op. L3 runs one matmul per chunk PAIR: the two-deep h2 column block
    [128,128] is exactly the stacked operand for a block-diagonal
    rhs [[Wo,0],[0,Wo]] [128,512], yielding hp for both chunks in one
    [128,512] PSUM bank (one LDWEIGHTS per two chunks). Odd slot sizes
    get a single-chunk tail matmul.
  - DVE multiplies each hp pair by xs (broadcast along o) in ONE op
    straight out of PSUM -> prod bf16 [128, 2, 16, 16]. Scatter is
    transposed: acc2T[n, (o,i)] += oh_c^T @ prod_c -- one N=256 matmul
    and one LDWEIGHTS (the one-hot) per chunk, a single per-slot
    accumulation group in one PSUM bank. A per-slot DVE tensor_reduce
    over i produces out[n, o] directly; one [128, 49*16] DMA returns it.
    No collectives; host un-permutes slots, concatenates, adds the bias.
"""

import numpy as np
import ml_dtypes

N_POINTS = 50000
N_PAD = 50176          # 8 * 49 * 128
N_CORES = 8
BUCKET = 128           # nodes per bucket
B_PER_CORE = 49
N_BUCKETS = N_PAD // BUCKET   # 392
CORE_NODES = B_PER_CORE * BUCKET  # 6272
IN_CH = 16
OUT_CH = 16
HID = 64
POS = 3

BF16 = ml_dtypes.bfloat16

_PROGRAM_CACHE = {}


def _build_program(ks):
    """Build + compile the per-core Bass program.

    ks = tuple of 49 per-slot chunk counts (shared across cores)."""
    import concourse.bacc as bacc
    import concourse.tile as tile
    import concourse.mybir as mybir

    f32 = mybir.dt.float32
    bf16 = mybir.dt.bfloat16

    nblks = [(k + 1) // 2 for k in ks]       # 128-col blocks in two-deep layout
    hoffs = np.concatenate([[0], np.cumsum([n * 128 for n in nblks])])
    coffs = np.concatenate([[0], np.cumsum(ks)])
    total_half = int(hoffs[-1])
    total_chunks = int(coffs[-1])
    Kmax = max(ks)
    Hmax = max(nblks) * 128

    nc = bacc.Bacc("TRN2", target_bir_lowering=False, debug=False)

    PT = nc.dram_tensor("PT", [12, total_half], bf16, kind="ExternalInput")
    XSB = nc.dram_tensor("XSB", [128, total_chunks * 16], bf16,
                         kind="ExternalInput")
    OH = nc.dram_tensor("OH", [128, total_chunks * 128], bf16,
                        kind="ExternalInput")
    W12 = nc.dram_tensor("W12", [128, HID], bf16, kind="ExternalInput")
    B1 = nc.dram_tensor("B1", [128, 1], f32, kind="ExternalInput")
    WH2 = nc.dram_tensor("WH2", [128, HID], bf16, kind="ExternalInput")
    BH = nc.dram_tensor("BH", [128, 1], f32, kind="ExternalInput")
    WOD = nc.dram_tensor("WOD", [128, 512], bf16, kind="ExternalInput")
    OUT = nc.dram_tensor("OUT", [128, B_PER_CORE * 16], f32,
                         kind="ExternalOutput")

    Gelu = mybir.ActivationFunctionType.Gelu
    MUL = mybir.AluOpType.mult
    ADD = mybir.AluOpType.add
    X = mybir.AxisListType.X

    with tile.TileContext(nc) as tc:
        with tc.tile_pool(name="const", bufs=1) as cp, \
             tc.tile_pool(name="io", bufs=2) as io, \
             tc.tile_pool(name="hh", bufs=2) as hh, \
             tc.tile_pool(name="wk", bufs=6) as wk, \
             tc.tile_pool(name="psMLP", bufs=2, space="PSUM") as psMLP, \
             tc.tile_pool(name="psH", bufs=4, space="PSUM") as psH, \
             tc.tile_pool(name="psAcc", bufs=2, space="PSUM") as psAcc:

            # --- constants ---
            w12_t = cp.tile([128, HID], bf16)
            nc.sync.dma_start(out=w12_t[:], in_=W12[:])
            b1_t = cp.tile([128, 1], f32)
            nc.sync.dma_start(out=b1_t[:], in_=B1[:])
            wh2_t = cp.tile([128, HID], bf16)
            nc.sync.dma_start(out=wh2_t[:], in_=WH2[:])
            bh_t = cp.tile([128, 1], f32)
            nc.sync.dma_start(out=bh_t[:], in_=BH[:])
            wod_t = cp.tile([128, 512], bf16)
            nc.sync.dma_start(out=wod_t[:], in_=WOD[:])

            fin_t = cp.tile([128, B_PER_CORE * 16], f32)

            for b in range(B_PER_CORE):
                k = ks[b]
                npairs, tail = k // 2, k % 2
                nblk = npairs + tail
                Hb = nblk * 128
                ho = int(hoffs[b])
                co = int(coffs[b])

                pt_t = io.tile([128, Hmax], bf16, tag="pt")
                nc.sync.dma_start(out=pt_t[0:6, 0:Hb],
                                  in_=PT[0:6, ho:ho + Hb])
                nc.sync.dma_start(out=pt_t[64:70, 0:Hb],
                                  in_=PT[6:12, ho:ho + Hb])
                xsb_t = io.tile([128, Kmax, 16], bf16, tag="xsb")
                nc.sync.dma_start(out=xsb_t[:, 0:k, :],
                                  in_=XSB[:, co * 16:(co + k) * 16])
                oh_t = io.tile([128, Kmax * 128], bf16, tag="oh")
                nc.sync.dma_start(out=oh_t[:, 0:k * 128],
                                  in_=OH[:, co * 128:(co + k) * 128])

                # ---- L1: two concurrent col-group tiles, gelu on 128p ----
                h1_t = hh.tile([128, Hmax], bf16, tag="h1")
                h2_t = hh.tile([128, Hmax], bf16, tag="h2")
                for s0 in range(0, Hb, 512):
                    w = min(512, Hb - s0)
                    p1 = psMLP.tile([128, 512], f32, tag="ps_mlp")
                    nc.tensor.matmul(p1[0:64, 0:w], lhsT=w12_t[0:6, :],
                                     rhs=pt_t[0:6, s0:s0 + w],
                                     start=True, stop=True)
                    nc.tensor.matmul(p1[64:128, 0:w], lhsT=w12_t[64:70, :],
                                     rhs=pt_t[64:70, s0:s0 + w],
                                     start=True, stop=True)
                    nc.scalar.activation(h1_t[:, s0:s0 + w], p1[:, 0:w], Gelu,
                                         bias=b1_t[:], scale=1.0)
                # ---- L2 slices emitted lazily inside the pair loop ----
                def emit_l2(s0):
                    w = min(512, Hb - s0)
                    p2 = psMLP.tile([128, 512], f32, tag="ps_mlp")
                    nc.tensor.matmul(p2[0:64, 0:w], lhsT=wh2_t[0:64, :],
                                     rhs=h1_t[0:64, s0:s0 + w],
                                     start=True, stop=True)
                    nc.tensor.matmul(p2[64:128, 0:w], lhsT=wh2_t[64:128, :],
                                     rhs=h1_t[64:128, s0:s0 + w],
                                     start=True, stop=True)
                    nc.scalar.activation(h2_t[:, s0:s0 + w], p2[:, 0:w], Gelu,
                                         bias=bh_t[:], scale=1.0)
                l2_next = [0]

                # ---- per-pair hp (block-diag) + mult + skewed scatter ----
                acc2 = psAcc.tile([128, 256], f32, tag="acc2")
                pend = []

                def emit_scatter(chunks, prod2, first, last):
                    pr = prod2[:].rearrange("p c o i -> p (c o i)")
                    for idx, c in enumerate(chunks):
                        nc.tensor.matmul(
                            acc2[:], lhsT=oh_t[:, c * 128:(c + 1) * 128],
                            rhs=pr[:, idx * 256:(idx + 1) * 256],
                            start=(first and idx == 0),
                            stop=(last and idx == len(chunks) - 1))

                for m in range(nblk):
                    # h2 cols [m*128, (m+1)*128) need L2 slice m//4 (+1 ahead)
                    while l2_next[0] < Hb and l2_next[0] <= (m + 1) * 128:
                        emit_l2(l2_next[0])
                        l2_next[0] += 512
                    is_tail = (tail == 1 and m == nblk - 1)
                    hpP = psH.tile([128, 512], f32, tag="hpP")
                    prod2 = wk.tile([128, 2, 16, 16], bf16, tag="prod")
                    if not is_tail:
                        nc.tensor.matmul(hpP[:],
                                         lhsT=h2_t[:, m * 128:(m + 1) * 128],
                                         rhs=wod_t[:], start=True, stop=True)
                        xs_b = xsb_t[:, 2 * m:2 * m + 2, :].unsqueeze(
                            2).to_broadcast([128, 2, 16, 16])
                        nc.vector.tensor_tensor(
                            out=prod2[:],
                            in0=hpP[:].rearrange("p (c o i) -> p c o i",
                                                 c=2, o=16, i=16),
                            in1=xs_b, op=MUL)
                        chunks = [2 * m, 2 * m + 1]
                    else:
                        nc.tensor.matmul(hpP[:, 0:256],
                                         lhsT=h2_t[0:64, m * 128:(m + 1) * 128],
                                         rhs=wod_t[0:64, 0:256],
                                         start=True, stop=True)
                        xs_b = xsb_t[:, k - 1:k, :].unsqueeze(
                            2).to_broadcast([128, 1, 16, 16])
                        nc.vector.tensor_tensor(
                            out=prod2[:, 0:1],
                            in0=hpP[:, 0:256].rearrange(
                                "p (c o i) -> p c o i", c=1, o=16, i=16),
                            in1=xs_b, op=MUL)
                        chunks = [k - 1]
                    if len(pend) == 5:
                        e = pend.pop(0)
                        emit_scatter(e[0], e[1], e[2], False)
                    pend.append((chunks, prod2, m == 0))
                for j, e in enumerate(pend):
                    emit_scatter(e[0], e[1], e[2], j == len(pend) - 1)

                # out[n, o] = sum_i acc2T[n, (o,i)]
                nc.vector.tensor_reduce(
                    out=fin_t[:, b * 16:(b + 1) * 16],
                    in_=acc2[:].rearrange("p (o i) -> p o i", o=16, i=16),
                    axis=X, op=ADD)

            nc.sync.dma_start(out=OUT[:], in_=fin_t[:])

    nc.compile()
    return nc


def _host_prep(x, pos, edge_index, W1, b1, Wh, bh, Wo, bo):
    """Bin edges by dst bucket, sort buckets into slots, gather, pad."""
    x_flat = np.ascontiguousarray(x.reshape(-1, IN_CH).astype(np.float32))
    pos = np.ascontiguousarray(pos.astype(np.float32))
    src = np.asarray(edge_index[0], dtype=np.int64)
    dst = np.asarray(edge_index[1], dtype=np.int64)
    E = src.shape[0]

    bucket = (dst >> 7).astype(np.int64)          # 0..391
    order = np.argsort(bucket, kind="stable")     # edge ids sorted by bucket
    cnt = np.bincount(bucket, minlength=N_BUCKETS)
    starts = np.zeros(N_BUCKETS, dtype=np.int64)
    starts[1:] = np.cumsum(cnt)[:-1]

    cnt_pc = cnt.reshape(N_CORES, B_PER_CORE)
    perms = np.argsort(-cnt_pc, axis=1, kind="stable")   # slot -> local bucket
    sorted_cnt = np.take_along_axis(cnt_pc, perms, axis=1)
    slot_max = sorted_cnt.max(axis=0)
    ks = tuple(int(v) for v in np.maximum(1, np.ceil(slot_max / 128))
               .astype(np.int64))

    nblks = [(k + 1) // 2 for k in ks]
    hoffs = np.concatenate([[0], np.cumsum([n * 128 for n in nblks])])
    coffs = np.concatenate([[0], np.cumsum(ks)])
    total_half = int(hoffs[-1])
    total_chunks = int(coffs[-1])

    # host-side bias term
    e_src_all = src[order]
    e_dst_all = dst[order]
    xs_agg = np.zeros((N_PAD, IN_CH), dtype=np.float32)
    np.add.at(xs_agg, e_dst_all, x_flat[e_src_all])
    bo16 = np.asarray(bo, dtype=np.float32).reshape(IN_CH, OUT_CH)
    bias_full = xs_agg @ bo16                      # [N_PAD, 16]

    per_core = []
    for c in range(N_CORES):
        PT2 = np.zeros((12, total_half), dtype=np.float32)
        XS2 = np.zeros((128, total_chunks, 16), dtype=np.float32)
        OH2 = np.zeros((128, total_chunks, 128), dtype=BF16)
        for b in range(B_PER_CORE):
            k = ks[b]
            nblk = nblks[b]
            ho = int(hoffs[b])
            g = c * B_PER_CORE + int(perms[c][b])  # global bucket id
            n = int(cnt_pc[c][perms[c][b]])
            if n == 0:
                continue
            eids = order[starts[g]:starts[g] + n]
            es, ed = src[eids], dst[eids]
            pe6 = np.concatenate([pos[es], pos[ed]], axis=1)   # [n, 6]
            xse = x_flat[es]                                   # [n, 16]
            dl = (ed - (g << 7)).astype(np.int64)
            ch = np.arange(n) // 128                           # chunk in slot
            rw = np.arange(n) % 128                            # row (edge lane)
            # PT two-deep: chunk 2m -> rows 0-5 block m; 2m+1 -> rows 6-11
            colh = (ch // 2) * 128 + rw
            hi = (ch % 2) * 6
            PT2[hi, ho + colh] = pe6[:, 0]
            PT2[hi + 1, ho + colh] = pe6[:, 1]
            PT2[hi + 2, ho + colh] = pe6[:, 2]
            PT2[hi + 3, ho + colh] = pe6[:, 3]
            PT2[hi + 4, ho + colh] = pe6[:, 4]
            PT2[hi + 5, ho + colh] = pe6[:, 5]
            XS2[rw, int(coffs[b]) + ch] = xse
            OH2[rw, int(coffs[b]) + ch, dl] = 1
        per_core.append({
            "PT": PT2.astype(BF16),
            "XSB": np.ascontiguousarray(XS2.reshape(128, total_chunks * 16)
                                        ).astype(BF16),
            "OH": np.ascontiguousarray(OH2.reshape(128, total_chunks * 128)),
        })

    # weights (shared across cores)
    W1a = np.asarray(W1, dtype=BF16)                                # [6, 64]
    W12 = np.zeros((128, HID), dtype=BF16)
    W12[0:6] = W1a
    W12[64:70] = W1a
    b1a = np.tile(np.asarray(b1, dtype=np.float32).reshape(HID, 1), (2, 1))
    Wha = np.asarray(Wh, dtype=BF16)                                # [64, 64]
    Wh2 = np.vstack([Wha, Wha])                                     # [128, 64]
    bha = np.tile(np.asarray(bh, dtype=np.float32).reshape(HID, 1), (2, 1))
    WoP = np.asarray(Wo, dtype=np.float32).reshape(HID, IN_CH, OUT_CH)
    WoP = np.ascontiguousarray(WoP.transpose(0, 2, 1)).reshape(HID, 256)
    WoP = WoP.astype(BF16)                                          # [64,(o,i)]
    WoD = np.zeros((128, 512), dtype=BF16)
    WoD[0:64, 0:256] = WoP
    WoD[64:128, 256:512] = WoP
    shared = {"W12": W12, "B1": b1a, "WH2": Wh2, "BH": bha, "WOD": WoD}
    for m in per_core:
        m.update(shared)
    return ks, perms, per_core, bias_full


def kernel(**inputs):
    from concourse import bass_utils

    ks, perms, in_maps, bias_full = _host_prep(
        inputs["x"], inputs["pos"], inputs["edge_index"],
        inputs["W1"], inputs["b1"], inputs["Wh"], inputs["bh"],
        inputs["Wo"], inputs["bo"])

    if ks not in _PROGRAM_CACHE:
        _PROGRAM_CACHE[ks] = _build_program(ks)
    nc = _PROGRAM_CACHE[ks]

    res = bass_utils.run_bass_kernel_spmd(nc, in_maps,
                                          core_ids=list(range(N_CORES)))
    cores = []
    for c, r in enumerate(res.results):
        o = r["OUT"]                                   # [128, 49*16] slot-major
        o = o.reshape(128, B_PER_CORE, OUT_CH).transpose(1, 0, 2)
        core_out = np.empty((B_PER_CORE, 128, OUT_CH), dtype=np.float32)
        core_out[perms[c]] = o                         # un-permute slots
        cores.append(core_out.reshape(CORE_NODES, OUT_CH))
    out = np.concatenate(cores, axis=0)                # [50176, 16]
    out = out + bias_full
    return np.ascontiguousarray(
        out[:N_POINTS].reshape(1, N_POINTS, OUT_CH).astype(np.float32))
